# revision 26
# baseline (speedup 1.0000x reference)
"""GQA causal attention with RoPE, tensor-parallel over heads on 8 TRN2 NeuronCores.

Reference computation (per problem spec, all f32):
  q = rope(x @ Wq), k = rope(x @ Wk), v = x @ Wv    (GQA: 32 q heads, 8 kv heads, hd=64)
  out = softmax(causal(q k^T / 8)) v @ Wo

Sharding: core c owns q-heads 4c..4c+3 and kv-head c (column shards of
Wq/Wk/Wv).  Attention outputs (kept transposed, feature-major) are
AllGathered per 512-token chunk; the Wo projection is column-split: core c
computes out[:, 256c:256(c+1)] with the full gathered activations, so the
final output assembles by concatenation with no AllReduce.

The host pre-packs everything the device would otherwise shuffle: x is
transposed/bf16-cast/chunk-packed on the host (no on-device DMA
transposes), RoPE tables arrive in their final [128, S] layout, and the
rotate-half permutation / transpose-identity matrices are host constants.

Layout trick: scores are computed transposed (S^T = K Q^T, keys on
partitions, queries free) so the exp'd scores feed the PV matmul directly
as the moving operand.  A ones-column appended to V yields the softmax
denominators in the same PV matmul.  Fully-masked 128x128 causal blocks
are never computed (score matmuls are narrowed on the moving side).

Pipeline: 8 chunks of 512 tokens flow through proj -> attention ->
AllGather -> Wo with the collective for chunk k hidden behind compute of
chunk k+1.
"""

import os
import sys

import numpy as np

for _p in ("/opt/trn_rl_repo",):
    if os.path.isdir(_p) and _p not in sys.path:
        sys.path.insert(0, _p)

from contextlib import ExitStack

import ml_dtypes

import concourse.bass as bass
import concourse.tile as tile
from concourse import bacc, mybir
from concourse.bass_utils import run_bass_kernel_spmd

B, S, HID = 2, 2048, 2048
NH, NKV, HD = 32, 8, 64
TP = 8
QH = NH // TP          # 4 q heads per core
T = B * S              # 4096 tokens
QF = QH * HD           # 256 q features per core
OC = HID // TP         # 256 out cols per core
TOKC = 512             # tokens per chunk
NHB = HID // 128       # 16 hid blocks
NCH = B * (S // TOKC)  # 8 chunks total

F32 = mybir.dt.float32
BF = mybir.dt.bfloat16
BF_NP = ml_dtypes.bfloat16

LAST_RESULTS = None
_NC_CACHE = None


def build_nc():
    nc = bacc.Bacc(None, target_bir_lowering=False)

    xt = nc.declare_dram_parameter("xt", [NCH * 128, NHB, TOKC], BF, False)
    wq = nc.declare_dram_parameter("wq", [128, NHB, QF], BF, False)
    wkv = nc.declare_dram_parameter("wkv", [128, NHB, 128], BF, False)
    wo = nc.declare_dram_parameter("wo", [128, NHB, OC], BF, False)
    cosT = nc.declare_dram_parameter("cosT", [128, S], BF, False)
    sinTs = nc.declare_dram_parameter("sinTs", [128, S], BF, False)
    mrot = nc.declare_dram_parameter("mrot", [128, 128], BF, False)
    idhi = nc.declare_dram_parameter("idhi", [128, HD], BF, False)
    # 4 causal masks for diagonal key-blocks: cmask[:, d*512:(d+1)*512][r, c]
    # = 1 where c >= 128*d + r else 0
    cmask = nc.declare_dram_parameter("cmask", [128, 4 * TOKC], BF, False)
    out = nc.declare_dram_parameter("out", [OC, T], F32, isOutput=True)

    with tile.TileContext(nc) as tc, ExitStack() as ctx:
        const = ctx.enter_context(tc.tile_pool(name="const", bufs=1))
        dram = ctx.enter_context(tc.tile_pool(name="dram", bufs=1, space="DRAM"))

        # PSUM budget (8 banks): psum_s 2x2 + psum_a 3x1 + psum_w 1x1.
        # psum_w is wo-only so collective latency never stalls proj/attn allocs.
        psum_s = ctx.enter_context(tc.tile_pool(name="psum_s", bufs=2, space="PSUM"))
        psum_a = ctx.enter_context(tc.tile_pool(name="psum_a", bufs=3, space="PSUM"))
        psum_w = ctx.enter_context(tc.tile_pool(name="psum_w", bufs=1, space="PSUM"))

        # ---- constants / weights (single packed DMA each) -------------
        wq_pk = const.tile([128, NHB, QF], BF)
        nc.sync.dma_start(wq_pk[:], wq[:])
        wkv_pk = const.tile([128, NHB, 128], BF)
        nc.sync.dma_start(wkv_pk[:], wkv[:])
        wo_pk = const.tile([128, NHB, OC], BF)
        nc.sync.dma_start(wo_pk[:], wo[:])
        cosT_sb = const.tile([128, S], BF)
        nc.sync.dma_start(cosT_sb[:], cosT[:])
        sinTs_sb = const.tile([128, S], BF)
        nc.sync.dma_start(sinTs_sb[:], sinTs[:])
        Mrot = const.tile([128, 128], BF)
        nc.sync.dma_start(Mrot[:], mrot[:])
        id64hi = const.tile([128, HD], BF)
        nc.sync.dma_start(id64hi[:], idhi[:])
        cmask_sb = const.tile([128, 4 * TOKC], BF)
        nc.sync.dma_start(cmask_sb[:], cmask[:])
        onesb = const.tile([1, HD], BF)
        nc.vector.memset(onesb[:], 1.0)

        wq_sb = [wq_pk[:, hb, :] for hb in range(NHB)]
        wkv_sb = [wkv_pk[:, hb, :] for hb in range(NHB)]
        wo_sb = [wo_pk[:, hb, :] for hb in range(NHB)]

        # ---- collective buffers (per chunk) ---------------------------
        ag_in = [dram.tile([QF, TOKC], BF, name=f"agin{cn}") for cn in range(NCH)]
        ag_out = [dram.tile([TP * QF, TOKC], BF, addr_space="Shared",
                            name=f"agout{cn}") for cn in range(NCH)]

        # ---- pools ----------------------------------------------------
        xa_pool = ctx.enter_context(tc.tile_pool(name="xa", bufs=2))
        qkv_pool = ctx.enter_context(tc.tile_pool(name="qkv", bufs=2))
        rope_pool = ctx.enter_context(tc.tile_pool(name="rope", bufs=2))
        v_pool = ctx.enter_context(tc.tile_pool(name="vtile", bufs=2 * (S // 128)))
        e_pool = ctx.enter_context(tc.tile_pool(name="epool", bufs=9))
        r_pool = ctx.enter_context(tc.tile_pool(name="rpool", bufs=4))
        at_pool = ctx.enter_context(tc.tile_pool(name="atp", bufs=4))
        wo_sbp = ctx.enter_context(tc.tile_pool(name="ag_sb", bufs=2))
        wo_out = ctx.enter_context(tc.tile_pool(name="wo_o", bufs=2))

        qts = {}
        kvTs = {}
        kdups = {}
        vtss = {}

        def rope_tile(dst_ap, src_sb_ap, psr_ap, cs, hi):
            # dst = src*cos + (Mrot.T@src)*sinTs ; all [hi, TOKC]
            rot = rope_pool.tile([hi, TOKC], BF, tag="rot")
            nc.vector.tensor_mul(rot[:], psr_ap, sinTs_sb[0:hi, cs])
            tmp = rope_pool.tile([hi, TOKC], BF, tag="tmp")
            nc.vector.tensor_mul(tmp[:], src_sb_ap, cosT_sb[0:hi, cs])
            nc.vector.tensor_add(dst_ap, tmp[:], rot[:])

        def proj(cn):
            b, qc = cn // 4, cn % 4
            if qc == 0:
                qts[b] = [qkv_pool.tile([128, S], BF, tag=f"qt{i}",
                                        name=f"qt{b}_{i}") for i in range(2)]
                kvTs[b] = qkv_pool.tile([128, S], BF, tag="kvT", name=f"kvT{b}")
                kdups[b] = qkv_pool.tile([128, S], BF, tag="kdup", name=f"kdup{b}")
                vtss[b] = []
            qt, kvT, kdup, vts = qts[b], kvTs[b], kdups[b], vtss[b]
            cs = slice(qc * TOKC, (qc + 1) * TOKC)

            xtp = xa_pool.tile([128, NHB, TOKC], BF, tag="xt", name=f"xt{cn}")
            nc.sync.dma_start(xtp[:], xt[cn * 128:(cn + 1) * 128, :, :])

            # all 48 QKV matmuls back-to-back so PE never waits on the
            # PSUM-drain copies; rope matmuls follow once copies are done
            psq0 = psum_a.tile([128, TOKC], F32, tag="a", name=f"q0_{cn}")
            for hb in range(NHB):
                nc.tensor.matmul(psq0[:], wq_sb[hb][:, 0:128], xtp[:, hb, :],
                                 start=hb == 0, stop=hb == NHB - 1)
            nc.scalar.copy(qt[0][:, cs], psq0[:])
            psq1 = psum_a.tile([128, TOKC], F32, tag="a", name=f"q1_{cn}")
            for hb in range(NHB):
                nc.tensor.matmul(psq1[:], wq_sb[hb][:, 128:256], xtp[:, hb, :],
                                 start=hb == 0, stop=hb == NHB - 1)
            nc.scalar.copy(qt[1][:, cs], psq1[:])
            pskv = psum_a.tile([128, TOKC], F32, tag="a", name=f"kv_{cn}")
            for hb in range(NHB):
                nc.tensor.matmul(pskv[:], wkv_sb[hb], xtp[:, hb, :],
                                 start=hb == 0, stop=hb == NHB - 1)
            nc.scalar.copy(kvT[:, cs], pskv[:])

            psR0 = psum_a.tile([128, TOKC], F32, tag="a", name=f"pr0_{cn}")
            nc.tensor.matmul(psR0[:], Mrot[:], qt[0][:, cs], start=True, stop=True)
            psR1 = psum_a.tile([128, TOKC], F32, tag="a", name=f"pr1_{cn}")
            nc.tensor.matmul(psR1[:], Mrot[:], qt[1][:, cs], start=True, stop=True)
            psRk = psum_a.tile([HD, TOKC], F32, tag="a", name=f"prk_{cn}")
            nc.tensor.matmul(psRk[:], Mrot[0:HD, 0:HD], kvT[0:HD, cs],
                             start=True, stop=True)
            # V token-major tiles (ones column appended for denominators)
            psvs = []
            for vb in range(qc * 4, qc * 4 + 4):
                psv = psum_a.tile([128, HD], BF, tag="a", name=f"vps{b}_{vb}")
                nc.tensor.transpose(psv[:], kvT[HD:128, vb * 128:(vb + 1) * 128],
                                    id64hi[HD:128, :])
                psvs.append(psv)

            rope_tile(qt[0][:, cs], qt[0][:, cs], psR0[:], cs, 128)
            rope_tile(qt[1][:, cs], qt[1][:, cs], psR1[:], cs, 128)
            rope_tile(kvT[0:HD, cs], kvT[0:HD, cs], psRk[:], cs, HD)
            # duplicate roped K^T to partitions 64:128 for odd heads
            # (scalar queue: the sync queue must stay free for xtp prefetch)
            nc.scalar.dma_start(kdup[HD:128, cs], kvT[0:HD, cs])
            for vi, vb in enumerate(range(qc * 4, qc * 4 + 4)):
                vt_ = v_pool.tile([128, HD + 1], BF, tag="vt", name=f"vt{b}_{vb}")
                nc.scalar.copy(vt_[:, 0:HD], psvs[vi][:])
                nc.vector.memset(vt_[:, HD:HD + 1], 1.0)
                vts.append(vt_)

        def finalize(cn, h, psO):
            # softmax denominators -> reciprocal -> PE broadcast -> scale.
            # Deferred one head so the PE queue never stalls on the DVE chain.
            # (reciprocal_approx_fast must read from SBUF, not PSUM)
            srow = r_pool.tile([1, TOKC], F32, tag="sr", name=f"sr{cn}{h}")
            nc.vector.tensor_copy(srow[:], psO[HD:HD + 1, :])
            recd = r_pool.tile([1, TOKC], F32, tag="rd", name=f"rd{cn}{h}")
            nc.vector.reciprocal_approx_fast(recd[:], srow[:])
            recb = r_pool.tile([1, TOKC], BF, tag="rb", name=f"rb{cn}{h}")
            nc.vector.tensor_copy(recb[:], recd[:])
            psB = psum_s.tile([HD, TOKC], F32, tag="s", name=f"psB{cn}{h}")
            nc.tensor.matmul(psB[:], onesb[0:1, :], recb[:],
                             start=True, stop=True)
            bcs = r_pool.tile([HD, TOKC], BF, tag="bcs", name=f"bc{cn}{h}")
            nc.scalar.copy(bcs[:], psB[:])
            at = at_pool.tile([HD, TOKC], BF, tag="at", name=f"at{cn}{h}")
            nc.vector.scalar_tensor_tensor(
                at[:], psO[0:HD, :], 1.0, bcs[:],
                mybir.AluOpType.bypass, mybir.AluOpType.mult)
            nc.scalar.dma_start(ag_in[cn][h * HD:(h + 1) * HD, :], at[:])

        def attn(cn):
            b, qc = cn // 4, cn % 4
            qt, kvT, kdup, vts = qts[b], kvTs[b], kdups[b], vtss[b]
            nkb = (qc + 1) * 4
            pend = None
            for h in range(QH):
                r = h % 2
                qh_ap = qt[h // 2][r * 64:r * 64 + 64, :]
                k_src = kvT if r == 0 else kdup
                es = []  # (tile, col offset, causal col offset) per kb
                psO = psum_a.tile([HD + 1, TOKC], F32, tag="a",
                                  name=f"psO{cn}{h}")

                def pv_group(g):
                    # PV for score-group g; interleaved one group behind the
                    # score matmuls so PE has work while ACT runs exp.
                    # Diagonal key-blocks contribute nothing to queries left
                    # of the diagonal, so the moving operand is narrowed and
                    # only a 128-wide triangle needs masking.
                    for kb in (2 * g, 2 * g + 1):
                        e, off, o = es[kb]
                        nc.tensor.matmul(
                            psO[:, o:], vts[kb][:],
                            e[:, off + o:off + TOKC],
                            start=(kb == 0), stop=(kb == nkb - 1),
                            skip_group_check=True)

                for g in range(nkb // 2):
                    psS = psum_s.tile([128, 1024], F32, tag="s",
                                      name=f"psS{cn}{h}_{g}")
                    e = e_pool.tile([128, 1024], BF, tag="e",
                                    name=f"e{cn}{h}_{g}")
                    o0 = TOKC
                    for j in range(2):
                        kb = 2 * g + j
                        o = max(0, kb * 128 - qc * TOKC)
                        o0 = min(o0, o)
                        nc.tensor.matmul(
                            psS[:, j * TOKC + o:(j + 1) * TOKC],
                            k_src[r * 64:r * 64 + 64, kb * 128:(kb + 1) * 128],
                            qh_ap[:, qc * TOKC + o:(qc + 1) * TOKC],
                            start=True, stop=True)
                        es.append((e, j * TOKC, o))
                    nc.scalar.activation(
                        e[:, o0:], psS[:, o0:], mybir.ActivationFunctionType.Exp,
                        scale=0.125)
                    for j in range(2):
                        kb = 2 * g + j
                        if kb >= nkb - 4:
                            o = es[kb][2]
                            nc.vector.tensor_mul(
                                e[:, j * TOKC + o:j * TOKC + o + 128],
                                e[:, j * TOKC + o:j * TOKC + o + 128],
                                cmask_sb[:, 0:128])
                    if g >= 1:
                        pv_group(g - 1)
                pv_group(nkb // 2 - 1)
                if pend is not None:
                    finalize(cn, *pend)
                pend = (h, psO)
            return pend

        def ag(cn):
            nc.gpsimd.collective_compute(
                "AllGather", mybir.AluOpType.bypass,
                ins=[ag_in[cn][:].opt()], outs=[ag_out[cn][:].opt()],
                replica_groups=[list(range(TP))],
            )

        def wo_chunk(cn):
            agt = wo_sbp.tile([128, NHB, TOKC], BF, tag="agt", name=f"agt{cn}")
            for fb in range(NHB):
                nc.scalar.dma_start(agt[:, fb, :],
                                    ag_out[cn][fb * 128:(fb + 1) * 128, :])
            col = (cn // 4) * S + (cn % 4) * TOKC
            for mb in range(OC // 128):
                psW = psum_w.tile([128, TOKC], F32, tag="w", name=f"psW{cn}_{mb}")
                for fb in range(NHB):
                    nc.tensor.matmul(
                        psW[:], wo_sb[fb][:, mb * 128:(mb + 1) * 128],
                        agt[:, fb, :], start=(fb == 0), stop=(fb == NHB - 1))
                osb = wo_out.tile([128, TOKC], F32, tag="osb",
                                  name=f"osb{cn}_{mb}")
                nc.vector.tensor_copy(osb[:], psW[:])
                nc.scalar.dma_start(
                    out[mb * 128:(mb + 1) * 128, col:col + TOKC], osb[:])

        for cn in range(NCH):
            proj(cn)
            pend = attn(cn)
            if cn >= 2:
                wo_chunk(cn - 2)
            finalize(cn, *pend)
            ag(cn)
        wo_chunk(NCH - 2)
        wo_chunk(NCH - 1)

    nc.compile()
    return nc


def _pack_inputs(inputs):
    x = np.asarray(inputs["x"], np.float32)
    cos = np.asarray(inputs["cos"], np.float32)
    sin = np.asarray(inputs["sin"], np.float32)
    Wq = np.asarray(inputs["Wq"], np.float32)
    Wk = np.asarray(inputs["Wk"], np.float32)
    Wv = np.asarray(inputs["Wv"], np.float32)
    Wo = np.asarray(inputs["Wo"], np.float32)

    # x chunks: xt[b*4+qc, p, hb, t] = x[b, qc*512+t, hb*128+p]
    xr = x.reshape(B, S // TOKC, TOKC, NHB, 128)
    xt = np.ascontiguousarray(
        xr.transpose(0, 1, 4, 3, 2).reshape(NCH * 128, NHB, TOKC)).astype(BF_NP)

    ct = cos.T.astype(np.float32)                      # [64, S]
    cosT = np.vstack([ct, ct]).astype(BF_NP)
    st = sin.T.astype(np.float32)
    sts = np.vstack([-st[0:32], st[32:64]])
    sinTs = np.vstack([sts, sts]).astype(BF_NP)

    mrot = np.zeros((128, 128), np.float32)
    for o in (0, 64):
        for j in range(32):
            mrot[o + 32 + j, o + j] = 1.0
            mrot[o + j, o + 32 + j] = 1.0
    mrot = mrot.astype(BF_NP)
    idhi = np.zeros((128, HD), np.float32)
    for j in range(HD):
        idhi[64 + j, j] = 1.0
    idhi = idhi.astype(BF_NP)

    col = np.arange(TOKC)[None, :]
    row = np.arange(128)[:, None]
    cmask = np.concatenate(
        [(col >= 128 * d + row).astype(np.float32) for d in range(4)],
        axis=1).astype(BF_NP)

    in_maps = []
    for c in range(TP):
        wq_c = np.ascontiguousarray(
            Wq[:, c * QF:(c + 1) * QF].reshape(NHB, 128, QF)
            .transpose(1, 0, 2)).astype(BF_NP)
        wk_c = Wk[:, c * HD:(c + 1) * HD].reshape(NHB, 128, HD)
        wv_c = Wv[:, c * HD:(c + 1) * HD].reshape(NHB, 128, HD)
        wkv_c = np.ascontiguousarray(
            np.concatenate([wk_c, wv_c], axis=2).transpose(1, 0, 2)).astype(BF_NP)
        wo_c = np.ascontiguousarray(
            Wo[:, c * OC:(c + 1) * OC].reshape(NHB, 128, OC)
            .transpose(1, 0, 2)).astype(BF_NP)
        in_maps.append({
            "xt": xt, "cosT": cosT, "sinTs": sinTs, "mrot": mrot, "idhi": idhi,
            "cmask": cmask, "wq": wq_c, "wkv": wkv_c, "wo": wo_c,
        })
    return in_maps


def kernel(**inputs):
    global LAST_RESULTS, _NC_CACHE
    if _NC_CACHE is None:
        _NC_CACHE = build_nc()
    nc = _NC_CACHE

    in_maps = _pack_inputs(inputs)
    res = run_bass_kernel_spmd(nc, in_maps, core_ids=list(range(TP)))
    LAST_RESULTS = res
    full = np.concatenate([res.results[c]["out"] for c in range(TP)], axis=0).T
    return np.ascontiguousarray(full.reshape(B, S, HID), dtype=np.float32)


if __name__ == "__main__":
    nc = build_nc()
    print("build OK, instructions:",
          sum(len(bb.instructions) for bb in nc.main_func.blocks))


# revision 27
# speedup vs baseline: 1.1038x; 1.1038x over previous
"""GQA causal attention with RoPE, tensor-parallel over heads on 8 TRN2 NeuronCores.

Reference computation (per problem spec, all f32):
  q = rope(x @ Wq), k = rope(x @ Wk), v = x @ Wv    (GQA: 32 q heads, 8 kv heads, hd=64)
  out = softmax(causal(q k^T / 8)) v @ Wo

Sharding: core c owns q-heads 4c..4c+3 and kv-head c (column shards of
Wq/Wk/Wv).  Attention outputs (kept transposed, feature-major) are
AllGathered per 512-token chunk; the Wo projection is column-split: core c
computes out[:, 256c:256(c+1)] with the full gathered activations, so the
final output assembles by concatenation with no AllReduce.

The host pre-packs everything the device would otherwise shuffle: x is
transposed/bf16-cast/chunk-packed on the host (no on-device DMA
transposes), RoPE tables arrive in their final [128, S] layout, and the
rotate-half permutation / transpose-identity matrices are host constants.

Layout trick: scores are computed transposed (S^T = K Q^T, keys on
partitions, queries free) so the exp'd scores feed the PV matmul directly
as the moving operand.  A ones-column appended to V yields the softmax
denominators in the same PV matmul.  Fully-masked 128x128 causal blocks
are never computed (score matmuls are narrowed on the moving side).

Pipeline: 8 chunks of 512 tokens flow through proj -> attention ->
AllGather -> Wo with the collective for chunk k hidden behind compute of
chunk k+1.
"""

import os
import sys

import numpy as np

for _p in ("/opt/trn_rl_repo",):
    if os.path.isdir(_p) and _p not in sys.path:
        sys.path.insert(0, _p)

from contextlib import ExitStack

import ml_dtypes

import concourse.bass as bass
import concourse.tile as tile
from concourse import bacc, mybir
from concourse.bass_utils import run_bass_kernel_spmd

B, S, HID = 2, 2048, 2048
NH, NKV, HD = 32, 8, 64
TP = 8
QH = NH // TP          # 4 q heads per core
T = B * S              # 4096 tokens
QF = QH * HD           # 256 q features per core
OC = HID // TP         # 256 out cols per core
TOKC = 512             # tokens per chunk
NHB = HID // 128       # 16 hid blocks
NCH = B * (S // TOKC)  # 8 chunks total

F32 = mybir.dt.float32
BF = mybir.dt.bfloat16
BF_NP = ml_dtypes.bfloat16

LAST_RESULTS = None
_NC_CACHE = None


def build_nc():
    nc = bacc.Bacc(None, target_bir_lowering=False)

    xt = nc.declare_dram_parameter("xt", [NCH * 128, NHB, TOKC], BF, False)
    wq = nc.declare_dram_parameter("wq", [128, NHB, QF], BF, False)
    wkv = nc.declare_dram_parameter("wkv", [128, NHB, 128], BF, False)
    wo = nc.declare_dram_parameter("wo", [128, NHB, OC], BF, False)
    cosT = nc.declare_dram_parameter("cosT", [128, S], BF, False)
    sinTs = nc.declare_dram_parameter("sinTs", [128, S], BF, False)
    mrot = nc.declare_dram_parameter("mrot", [128, 128], BF, False)
    idhi = nc.declare_dram_parameter("idhi", [128, HD], BF, False)
    # 4 causal masks for diagonal key-blocks: cmask[:, d*512:(d+1)*512][r, c]
    # = 1 where c >= 128*d + r else 0
    cmask = nc.declare_dram_parameter("cmask", [128, 4 * TOKC], BF, False)
    out = nc.declare_dram_parameter("out", [OC, T], F32, isOutput=True)

    with tile.TileContext(nc) as tc, ExitStack() as ctx:
        const = ctx.enter_context(tc.tile_pool(name="const", bufs=1))
        dram = ctx.enter_context(tc.tile_pool(name="dram", bufs=1, space="DRAM"))

        # PSUM budget (8 banks): psum_s 2x2 + psum_a 3x1 + psum_w 1x1.
        # psum_w is wo-only so collective latency never stalls proj/attn allocs.
        psum_s = ctx.enter_context(tc.tile_pool(name="psum_s", bufs=2, space="PSUM"))
        psum_a = ctx.enter_context(tc.tile_pool(name="psum_a", bufs=3, space="PSUM"))
        psum_w = ctx.enter_context(tc.tile_pool(name="psum_w", bufs=1, space="PSUM"))

        # ---- constants / weights (single packed DMA each) -------------
        wq_pk = const.tile([128, NHB, QF], BF)
        nc.sync.dma_start(wq_pk[:], wq[:])
        wkv_pk = const.tile([128, NHB, 128], BF)
        nc.sync.dma_start(wkv_pk[:], wkv[:])
        wo_pk = const.tile([128, NHB, OC], BF)
        nc.sync.dma_start(wo_pk[:], wo[:])
        cosT_sb = const.tile([128, S], BF)
        nc.sync.dma_start(cosT_sb[:], cosT[:])
        sinTs_sb = const.tile([128, S], BF)
        nc.sync.dma_start(sinTs_sb[:], sinTs[:])
        Mrot = const.tile([128, 128], BF)
        nc.sync.dma_start(Mrot[:], mrot[:])
        id64hi = const.tile([128, HD], BF)
        nc.sync.dma_start(id64hi[:], idhi[:])
        cmask_sb = const.tile([128, 4 * TOKC], BF)
        nc.sync.dma_start(cmask_sb[:], cmask[:])
        onesb = const.tile([1, HD], BF)
        nc.vector.memset(onesb[:], 1.0)

        wq_sb = [wq_pk[:, hb, :] for hb in range(NHB)]
        wkv_sb = [wkv_pk[:, hb, :] for hb in range(NHB)]
        wo_sb = [wo_pk[:, hb, :] for hb in range(NHB)]

        # ---- collective buffers (per chunk) ---------------------------
        ag_in = [dram.tile([QF, TOKC], BF, name=f"agin{cn}") for cn in range(NCH)]
        ag_out = [dram.tile([TP * QF, TOKC], BF, addr_space="Shared",
                            name=f"agout{cn}") for cn in range(NCH)]

        # ---- pools ----------------------------------------------------
        xa_pool = ctx.enter_context(tc.tile_pool(name="xa", bufs=2))
        qkv_pool = ctx.enter_context(tc.tile_pool(name="qkv", bufs=2))
        rope_pool = ctx.enter_context(tc.tile_pool(name="rope", bufs=2))
        v_pool = ctx.enter_context(tc.tile_pool(name="vtile", bufs=2 * (S // 128)))
        e_pool = ctx.enter_context(tc.tile_pool(name="epool", bufs=9))
        r_pool = ctx.enter_context(tc.tile_pool(name="rpool", bufs=4))
        at_pool = ctx.enter_context(tc.tile_pool(name="atp", bufs=4))
        wo_sbp = ctx.enter_context(tc.tile_pool(name="ag_sb", bufs=2))
        wo_out = ctx.enter_context(tc.tile_pool(name="wo_o", bufs=2))

        qts = {}
        kvTs = {}
        kdups = {}
        vtss = {}

        def rope_tile(dst_ap, src_sb_ap, psr_ap, cs, hi):
            # dst = src*cos + (Mrot.T@src)*sinTs ; all [hi, TOKC]
            rot = rope_pool.tile([hi, TOKC], BF, tag="rot")
            nc.vector.tensor_mul(rot[:], psr_ap, sinTs_sb[0:hi, cs])
            tmp = rope_pool.tile([hi, TOKC], BF, tag="tmp")
            nc.vector.tensor_mul(tmp[:], src_sb_ap, cosT_sb[0:hi, cs])
            nc.vector.tensor_add(dst_ap, tmp[:], rot[:])

        def proj(cn):
            b, qc = cn // 4, cn % 4
            if qc == 0:
                qts[b] = [qkv_pool.tile([128, S], BF, tag=f"qt{i}",
                                        name=f"qt{b}_{i}") for i in range(2)]
                kvTs[b] = qkv_pool.tile([128, S], BF, tag="kvT", name=f"kvT{b}")
                kdups[b] = qkv_pool.tile([128, S], BF, tag="kdup", name=f"kdup{b}")
                vtss[b] = []
            qt, kvT, kdup, vts = qts[b], kvTs[b], kdups[b], vtss[b]
            cs = slice(qc * TOKC, (qc + 1) * TOKC)

            xtp = xa_pool.tile([128, NHB, TOKC], BF, tag="xt", name=f"xt{cn}")
            nc.sync.dma_start(xtp[:], xt[cn * 128:(cn + 1) * 128, :, :])

            # all 48 QKV matmuls back-to-back so PE never waits on the
            # PSUM-drain copies; rope matmuls follow once copies are done
            psq0 = psum_a.tile([128, TOKC], F32, tag="a", name=f"q0_{cn}")
            for hb in range(NHB):
                nc.tensor.matmul(psq0[:], wq_sb[hb][:, 0:128], xtp[:, hb, :],
                                 start=hb == 0, stop=hb == NHB - 1)
            nc.scalar.copy(qt[0][:, cs], psq0[:])
            psq1 = psum_a.tile([128, TOKC], F32, tag="a", name=f"q1_{cn}")
            for hb in range(NHB):
                nc.tensor.matmul(psq1[:], wq_sb[hb][:, 128:256], xtp[:, hb, :],
                                 start=hb == 0, stop=hb == NHB - 1)
            nc.scalar.copy(qt[1][:, cs], psq1[:])
            pskv = psum_a.tile([128, TOKC], F32, tag="a", name=f"kv_{cn}")
            for hb in range(NHB):
                nc.tensor.matmul(pskv[:], wkv_sb[hb], xtp[:, hb, :],
                                 start=hb == 0, stop=hb == NHB - 1)
            nc.scalar.copy(kvT[:, cs], pskv[:])

            psR0 = psum_a.tile([128, TOKC], F32, tag="a", name=f"pr0_{cn}")
            nc.tensor.matmul(psR0[:], Mrot[:], qt[0][:, cs], start=True, stop=True)
            psR1 = psum_a.tile([128, TOKC], F32, tag="a", name=f"pr1_{cn}")
            nc.tensor.matmul(psR1[:], Mrot[:], qt[1][:, cs], start=True, stop=True)
            psRk = psum_a.tile([HD, TOKC], F32, tag="a", name=f"prk_{cn}")
            nc.tensor.matmul(psRk[:], Mrot[0:HD, 0:HD], kvT[0:HD, cs],
                             start=True, stop=True)
            # V token-major tiles (ones column appended for denominators)
            psvs = []
            for vb in range(qc * 4, qc * 4 + 4):
                psv = psum_a.tile([128, HD], BF, tag="a", name=f"vps{b}_{vb}")
                nc.tensor.transpose(psv[:], kvT[HD:128, vb * 128:(vb + 1) * 128],
                                    id64hi[HD:128, :])
                psvs.append(psv)

            rope_tile(qt[0][:, cs], qt[0][:, cs], psR0[:], cs, 128)
            rope_tile(qt[1][:, cs], qt[1][:, cs], psR1[:], cs, 128)
            rope_tile(kvT[0:HD, cs], kvT[0:HD, cs], psRk[:], cs, HD)
            # duplicate roped K^T to partitions 64:128 for odd heads
            # (scalar queue: the sync queue must stay free for xtp prefetch)
            nc.scalar.dma_start(kdup[HD:128, cs], kvT[0:HD, cs])
            for vi, vb in enumerate(range(qc * 4, qc * 4 + 4)):
                vt_ = v_pool.tile([128, HD + 1], BF, tag="vt", name=f"vt{b}_{vb}")
                nc.scalar.copy(vt_[:, 0:HD], psvs[vi][:])
                nc.vector.memset(vt_[:, HD:HD + 1], 1.0)
                vts.append(vt_)

        def finalize(cn, h, psO):
            # softmax denominators -> reciprocal -> PE broadcast -> scale.
            # Deferred one head so the PE queue never stalls on the DVE chain.
            # (reciprocal_approx_fast must read from SBUF, not PSUM)
            srow = r_pool.tile([1, TOKC], F32, tag="sr", name=f"sr{cn}{h}")
            nc.vector.tensor_copy(srow[:], psO[HD:HD + 1, :])
            recd = r_pool.tile([1, TOKC], F32, tag="rd", name=f"rd{cn}{h}")
            nc.vector.reciprocal_approx_fast(recd[:], srow[:])
            recb = r_pool.tile([1, TOKC], BF, tag="rb", name=f"rb{cn}{h}")
            nc.vector.tensor_copy(recb[:], recd[:])
            psB = psum_s.tile([HD, TOKC], F32, tag="s", name=f"psB{cn}{h}")
            nc.tensor.matmul(psB[:], onesb[0:1, :], recb[:],
                             start=True, stop=True)
            bcs = r_pool.tile([HD, TOKC], BF, tag="bcs", name=f"bc{cn}{h}")
            nc.scalar.copy(bcs[:], psB[:])
            at = at_pool.tile([HD, TOKC], BF, tag="at", name=f"at{cn}{h}")
            nc.vector.scalar_tensor_tensor(
                at[:], psO[0:HD, :], 1.0, bcs[:],
                mybir.AluOpType.bypass, mybir.AluOpType.mult)
            nc.scalar.dma_start(ag_in[cn][h * HD:(h + 1) * HD, :], at[:])

        def attn(cn):
            b, qc = cn // 4, cn % 4
            qt, kvT, kdup, vts = qts[b], kvTs[b], kdups[b], vtss[b]
            nkb = (qc + 1) * 4
            pend = None
            for h in range(QH):
                r = h % 2
                qh_ap = qt[h // 2][r * 64:r * 64 + 64, :]
                k_src = kvT if r == 0 else kdup
                es = []  # (tile, col offset, causal col offset) per kb
                psO = psum_a.tile([HD + 1, TOKC], F32, tag="a",
                                  name=f"psO{cn}{h}")

                def pv_group(g):
                    # PV for score-group g; interleaved one group behind the
                    # score matmuls so PE has work while ACT runs exp.
                    # Diagonal key-blocks contribute nothing to queries left
                    # of the diagonal, so the moving operand is narrowed and
                    # only a 128-wide triangle needs masking.
                    for kb in (2 * g, 2 * g + 1):
                        e, off, o = es[kb]
                        nc.tensor.matmul(
                            psO[:, o:], vts[kb][:],
                            e[:, off + o:off + TOKC],
                            start=(kb == 0), stop=(kb == nkb - 1),
                            skip_group_check=True)

                for g in range(nkb // 2):
                    psS = psum_s.tile([128, 1024], F32, tag="s",
                                      name=f"psS{cn}{h}_{g}")
                    e = e_pool.tile([128, 1024], BF, tag="e",
                                    name=f"e{cn}{h}_{g}")
                    o0 = TOKC
                    for j in range(2):
                        kb = 2 * g + j
                        o = max(0, kb * 128 - qc * TOKC)
                        o0 = min(o0, o)
                        nc.tensor.matmul(
                            psS[:, j * TOKC + o:(j + 1) * TOKC],
                            k_src[r * 64:r * 64 + 64, kb * 128:(kb + 1) * 128],
                            qh_ap[:, qc * TOKC + o:(qc + 1) * TOKC],
                            start=True, stop=True)
                        es.append((e, j * TOKC, o))
                    nc.scalar.activation(
                        e[:, o0:], psS[:, o0:], mybir.ActivationFunctionType.Exp,
                        scale=0.125)
                    for j in range(2):
                        kb = 2 * g + j
                        if kb >= nkb - 4:
                            o = es[kb][2]
                            nc.vector.tensor_mul(
                                e[:, j * TOKC + o:j * TOKC + o + 128],
                                e[:, j * TOKC + o:j * TOKC + o + 128],
                                cmask_sb[:, 0:128])
                    if g >= 1:
                        pv_group(g - 1)
                pv_group(nkb // 2 - 1)
                if pend is not None:
                    finalize(cn, *pend)
                pend = (h, psO)
            return pend

        def ag(cn):
            nc.gpsimd.collective_compute(
                "AllGather", mybir.AluOpType.bypass,
                ins=[ag_in[cn][:].opt()], outs=[ag_out[cn][:].opt()],
                replica_groups=[list(range(TP))],
            )

        def wo_chunk(cn):
            # gpsimd queue: these waits ride behind the collectives they
            # depend on, never blocking compute-engine queues
            agt = wo_sbp.tile([128, NHB, TOKC], BF, tag="agt", name=f"agt{cn}")
            for fb in range(NHB):
                nc.gpsimd.dma_start(agt[:, fb, :],
                                    ag_out[cn][fb * 128:(fb + 1) * 128, :])
            col = (cn // 4) * S + (cn % 4) * TOKC
            for mb in range(OC // 128):
                psW = psum_w.tile([128, TOKC], F32, tag="w", name=f"psW{cn}_{mb}")
                for fb in range(NHB):
                    nc.tensor.matmul(
                        psW[:], wo_sb[fb][:, mb * 128:(mb + 1) * 128],
                        agt[:, fb, :], start=(fb == 0), stop=(fb == NHB - 1))
                osb = wo_out.tile([128, TOKC], F32, tag="osb",
                                  name=f"osb{cn}_{mb}")
                nc.vector.tensor_copy(osb[:], psW[:])
                nc.scalar.dma_start(
                    out[mb * 128:(mb + 1) * 128, col:col + TOKC], osb[:])

        for cn in range(NCH):
            proj(cn)
            pend = attn(cn)
            if cn >= 2:
                wo_chunk(cn - 2)
            finalize(cn, *pend)
            ag(cn)
        wo_chunk(NCH - 2)
        wo_chunk(NCH - 1)

    nc.compile()
    return nc


def _pack_inputs(inputs):
    x = np.asarray(inputs["x"], np.float32)
    cos = np.asarray(inputs["cos"], np.float32)
    sin = np.asarray(inputs["sin"], np.float32)
    Wq = np.asarray(inputs["Wq"], np.float32)
    Wk = np.asarray(inputs["Wk"], np.float32)
    Wv = np.asarray(inputs["Wv"], np.float32)
    Wo = np.asarray(inputs["Wo"], np.float32)

    # x chunks: xt[b*4+qc, p, hb, t] = x[b, qc*512+t, hb*128+p]
    xr = x.reshape(B, S // TOKC, TOKC, NHB, 128)
    xt = np.ascontiguousarray(
        xr.transpose(0, 1, 4, 3, 2).reshape(NCH * 128, NHB, TOKC)).astype(BF_NP)

    ct = cos.T.astype(np.float32)                      # [64, S]
    cosT = np.vstack([ct, ct]).astype(BF_NP)
    st = sin.T.astype(np.float32)
    sts = np.vstack([-st[0:32], st[32:64]])
    sinTs = np.vstack([sts, sts]).astype(BF_NP)

    mrot = np.zeros((128, 128), np.float32)
    for o in (0, 64):
        for j in range(32):
            mrot[o + 32 + j, o + j] = 1.0
            mrot[o + j, o + 32 + j] = 1.0
    mrot = mrot.astype(BF_NP)
    idhi = np.zeros((128, HD), np.float32)
    for j in range(HD):
        idhi[64 + j, j] = 1.0
    idhi = idhi.astype(BF_NP)

    col = np.arange(TOKC)[None, :]
    row = np.arange(128)[:, None]
    cmask = np.concatenate(
        [(col >= 128 * d + row).astype(np.float32) for d in range(4)],
        axis=1).astype(BF_NP)

    in_maps = []
    for c in range(TP):
        wq_c = np.ascontiguousarray(
            Wq[:, c * QF:(c + 1) * QF].reshape(NHB, 128, QF)
            .transpose(1, 0, 2)).astype(BF_NP)
        wk_c = Wk[:, c * HD:(c + 1) * HD].reshape(NHB, 128, HD)
        wv_c = Wv[:, c * HD:(c + 1) * HD].reshape(NHB, 128, HD)
        wkv_c = np.ascontiguousarray(
            np.concatenate([wk_c, wv_c], axis=2).transpose(1, 0, 2)).astype(BF_NP)
        wo_c = np.ascontiguousarray(
            Wo[:, c * OC:(c + 1) * OC].reshape(NHB, 128, OC)
            .transpose(1, 0, 2)).astype(BF_NP)
        in_maps.append({
            "xt": xt, "cosT": cosT, "sinTs": sinTs, "mrot": mrot, "idhi": idhi,
            "cmask": cmask, "wq": wq_c, "wkv": wkv_c, "wo": wo_c,
        })
    return in_maps


def kernel(**inputs):
    global LAST_RESULTS, _NC_CACHE
    if _NC_CACHE is None:
        _NC_CACHE = build_nc()
    nc = _NC_CACHE

    in_maps = _pack_inputs(inputs)
    res = run_bass_kernel_spmd(nc, in_maps, core_ids=list(range(TP)))
    LAST_RESULTS = res
    full = np.concatenate([res.results[c]["out"] for c in range(TP)], axis=0).T
    return np.ascontiguousarray(full.reshape(B, S, HID), dtype=np.float32)


if __name__ == "__main__":
    nc = build_nc()
    print("build OK, instructions:",
          sum(len(bb.instructions) for bb in nc.main_func.blocks))
